# revision 29
# baseline (speedup 1.0000x reference)
"""Trainium2 Bass kernel for nn_AdjointConformTNN (gnn_message_passing).

Strategy (8-core pure data parallel over batch):
  - Host computes step=int(t*2), shards x and u[step] over 8 cores, pads each
    shard to 128*1024 elements. Element b lives at (partition p, group g),
    b = p*1024 + g.
  - Per core the MLP runs on the TensorEngine with 8 network copies packed
    block-diagonally per matmul (feature-major layout built by PE transposes).
  - ELU(z)+1 = exp(min(z,0)) + relu(z); sigmoid(y) = 0.5*tanh(y/2)+0.5 so all
    ACT calls stay in one table set (exp/tanh/relu/copy).
  - Adds after ELU/sigmoid are fused into accumulating PE transposes.
  - Message passing + finalization run batch-major with strided plane APs.
"""
import os
import sys
import numpy as np

for _p in ("/opt/trn_rl_repo", "/root/.axon_site/_ro/trn_rl_repo"):
    if os.path.isdir(_p) and _p not in sys.path:
        sys.path.insert(0, _p)

import concourse.bass as bass
import concourse.mybir as mybir
from concourse import tile
from concourse import bass_utils
from concourse.tile import ScopedClock


def _patched_drain_and_barrier(self, tick_clock, wait_clock):
    # This walrus build rejects >1 sync-wait on one Drain ("Too many sync
    # wait commands"); spread the kernel-tail waits over single-wait nops.
    nc = self.nc
    collector = nc.sync.nop()
    wait_clock.add_sem_waits(collector.ins, ScopedClock({None: tick_clock.global_clock}))
    si = collector.ins.sync_info
    waits = list(si.on_wait) if si is not None and si.on_wait else []
    upds = list(si.on_update) if si is not None and si.on_update else []
    if len(waits) > 1:
        collector.ins.sync_info = mybir.SyncInfo(on_wait=waits[:1], on_update=upds)
        for w in waits[1:]:
            n2 = nc.sync.nop()
            n2.ins.sync_info = mybir.SyncInfo(on_wait=[w], on_update=[])
    nc.sync.drain()
    nc.all_engine_barrier()
    popped = nc._tile_sem_poison_stack.pop()
    assert popped is self._sem_poison
    nc.clear_and_free_semaphores(list(self.sems.allocated().values()))
    nc.all_engine_barrier()


tile.TileContext._drain_and_barrier = _patched_drain_and_barrier

_MAX_WAITS = int(os.environ.get("KERNEL_MAX_WAITS", "1"))
_orig_lower_ordered = tile.TileContext._lower_ordered_insts


def _split_waits_in_list(nc, insts):
    out = []
    for inst in insts:
        si = getattr(inst, "sync_info", None)
        if (si is not None and si.on_wait and len(si.on_wait) > _MAX_WAITS
                and type(inst).__name__.startswith("Inst")):
            waits = list(si.on_wait)
            upds = list(si.on_update) if si.on_update else []
            for w in waits[:-_MAX_WAITS]:
                nop = mybir.InstNoOp(name=nc.get_next_instruction_name())
                nop.engine = inst.engine
                nop.sync_info = mybir.SyncInfo(on_wait=[w], on_update=[])
                out.append(nop)
            inst.sync_info = mybir.SyncInfo(on_wait=waits[-_MAX_WAITS:], on_update=upds)
        out.append(inst)
    return out


def _patched_lower_ordered(self, ordered):
    for k in list(ordered.keys()):
        ordered[k] = _split_waits_in_list(self.nc, ordered[k])
    return _orig_lower_ordered(self, ordered)


tile.TileContext._lower_ordered_insts = _patched_lower_ordered

F32 = mybir.dt.float32
ALU = mybir.AluOpType
AF = mybir.ActivationFunctionType

N_CORES = 8
B_TOTAL = 1_000_000
B_CORE = B_TOTAL // N_CORES          # 125000
G_TOT = 1024                          # groups per partition (padded)
NP_CORE = 128 * G_TOT                 # 131072 padded elements per core
N_SUPER = 4                           # super-chunks per core
G_SUPER = 256                         # groups per super-chunk
N_MACRO = 8                           # macro-chunks per super-chunk
G_MACRO = 32                          # groups per macro-chunk (4 blocks of 8)
NFEAT = 11                            # 7 u-features + 4 x-features
NSLOT = 8                             # network copies per matmul block
NHID = 6                              # 2 cond + 4 ploss hidden
NCOND = 15
VIRT = 4

_CACHE = {}
LAST_RESULTS = None  # BassKernelResults for test harness introspection
LAST_WALL = None


def _build_bass(ti, ai, tidc, ecap, ncu=NCOND):
    """Build the Bass program. ti/ai: [4,5] int index tables (host-known),
    tidc: temp_idcs values, ecap: exp(caps) floats, ncu: number of conducts
    actually referenced by adj_idx (the rest are never computed)."""
    nc = bass.Bass()

    u2_d = nc.dram_tensor("u2", [NP_CORE, 7], F32, kind="ExternalInput")
    x_d = nc.dram_tensor("xin", [NP_CORE, 4], F32, kind="ExternalInput")
    id_d = nc.dram_tensor("ident", [128, 128], F32, kind="ExternalInput")
    w1_d = nc.dram_tensor("w1blk", [NSLOT * NFEAT, NSLOT * NHID], F32, kind="ExternalInput")
    b1_d = nc.dram_tensor("b1blk", [NSLOT * NHID, 1], F32, kind="ExternalInput")
    w2c_d = nc.dram_tensor("w2cblk", [NSLOT * NHID, NSLOT * ncu], F32, kind="ExternalInput")
    b2c_d = nc.dram_tensor("b2cblk", [NSLOT * ncu, 1], F32, kind="ExternalInput")
    w2p_d = nc.dram_tensor("w2pblk", [NSLOT * NHID, NSLOT * VIRT], F32, kind="ExternalInput")
    b2ph_d = nc.dram_tensor("b2pblkh", [NSLOT * VIRT, 1], F32, kind="ExternalInput")
    out_d = nc.dram_tensor("out", [NP_CORE, 4], F32, kind="ExternalOutput")

    u2_v = u2_d[:].rearrange("(p g) f -> p g f", p=128)    # [128, 1024, 7]
    x_v = x_d[:].rearrange("(p g) f -> p g f", p=128)      # [128, 1024, 4]
    out_v = out_d[:].rearrange("(p g) f -> p g f", p=128)  # [128, 1024, 4]

    with tile.TileContext(nc) as tc:
        with (
            tc.tile_pool(name="const", bufs=1) as constp,
            tc.tile_pool(name="io", bufs=3) as iop,
            tc.tile_pool(name="fm", bufs=2) as fmp,
            tc.tile_pool(name="mp", bufs=2) as mpp,
            tc.tile_pool(name="ps_t4", bufs=2, space=bass.MemorySpace.PSUM) as ps_t4,
            tc.tile_pool(name="ps_h", bufs=1, space=bass.MemorySpace.PSUM) as ps_h,
            tc.tile_pool(name="ps_c", bufs=1, space=bass.MemorySpace.PSUM) as ps_c,
            tc.tile_pool(name="ps_p", bufs=1, space=bass.MemorySpace.PSUM) as ps_p,
            tc.tile_pool(name="ps_tc", bufs=2, space=bass.MemorySpace.PSUM) as ps_tc,
            tc.tile_pool(name="ps_tp", bufs=1, space=bass.MemorySpace.PSUM) as ps_tp,
        ):
            ident = constp.tile([128, 128], F32)
            w1 = constp.tile([NSLOT * NFEAT, NSLOT * NHID], F32)
            b1 = constp.tile([NSLOT * NHID, 1], F32)
            w2c = constp.tile([NSLOT * NHID, NSLOT * ncu], F32)
            b2c = constp.tile([NSLOT * ncu, 1], F32)
            w2p = constp.tile([NSLOT * NHID, NSLOT * VIRT], F32)
            b2ph = constp.tile([NSLOT * VIRT, 1], F32)
            def load_xu(s, parts=2):
                # finer first-part granularity lets the first transposes
                # start sooner (Tile tracks subregion deps)
                g0 = s * G_SUPER
                P = G_SUPER // parts
                xu = iop.tile([128, G_SUPER, NFEAT], F32, tag="xu")
                for h in range(parts):
                    nc.sync.dma_start(xu[:, h * P:(h + 1) * P, 0:7],
                                      u2_v[:, g0 + h * P:g0 + (h + 1) * P, :])
                    nc.sync.dma_start(xu[:, h * P:(h + 1) * P, 7:11],
                                      x_v[:, g0 + h * P:g0 + (h + 1) * P, :])
                return xu

            # ident first (gates first transpose); weights on the SWDGE
            # queue so they don't sit behind the bulk xu load on SP's ring
            nc.sync.dma_start(ident[:], id_d[:])
            xu0 = load_xu(0, parts=8)
            nc.gpsimd.dma_start(w1[:], w1_d[:])
            nc.gpsimd.dma_start(b1[:], b1_d[:])
            nc.gpsimd.dma_start(w2c[:], w2c_d[:])
            nc.gpsimd.dma_start(b2c[:], b2c_d[:])
            nc.gpsimd.dma_start(w2p[:], w2p_d[:])
            nc.gpsimd.dma_start(b2ph[:], b2ph_d[:])

            TOT = N_SUPER * N_MACRO
            st = {}    # per-macro pipeline state
            sup = {}   # per-super resources

            def ensure_super(s):
                if s in sup:
                    return
                xu_s = xu0 if s == 0 else load_xu(s)
                tcsb = mpp.tile([128, G_SUPER, ncu], F32, tag="tcsb")
                tpsb = mpp.tile([128, G_SUPER, VIRT], F32, tag="tpsb")
                sup[s] = {"xu": xu_s, "tcsb": tcsb, "tpsb": tpsb}

            def S0(mm):
                s, m = divmod(mm, N_MACRO)
                gm = m * G_MACRO
                xu = sup[s]["xu"]
                t4 = ps_t4.tile([NSLOT * NFEAT, 512], F32, tag="t4")
                for b in range(4):
                    nc.tensor.transpose(
                        t4[:, 128 * b:128 * (b + 1)],
                        xu[:, gm + 8 * b: gm + 8 * (b + 1), :],
                        ident[:],
                    )
                t4sb = fmp.tile([NSLOT * NFEAT, 512], F32, tag="t4sb")
                nc.scalar.copy(t4sb[:], t4[:])
                st[mm] = {"t4sb": t4sb}

            def S1(mm):
                h = ps_h.tile([NSLOT * NHID, 512], F32, tag="h")
                nc.tensor.matmul(h[:], w1[:], st[mm]["t4sb"][:])
                hsb = fmp.tile([NSLOT * NHID, 512], F32, tag="hsb")
                nc.scalar.activation(hsb[:], h[:], AF.Tanh, bias=b1[:])
                st[mm]["hsb"] = hsb

            def S2(mm):
                hsb = st[mm]["hsb"]
                cps = ps_c.tile([NSLOT * ncu, 512], F32, tag="cps")
                nc.tensor.matmul(cps[:], w2c[:], hsb[:])
                pps = ps_p.tile([NSLOT * VIRT, 512], F32, tag="pps")
                nc.tensor.matmul(pps[:], w2p[:], hsb[:])
                msb = fmp.tile([NSLOT * ncu, 512], F32, tag="msb")
                nc.vector.tensor_scalar(msb[:], cps[:], b2c[:], 0.0, ALU.add, ALU.min)
                esb = fmp.tile([NSLOT * ncu, 512], F32, tag="esb")
                nc.scalar.activation(esb[:], msb[:], AF.Exp)
                rsb = fmp.tile([NSLOT * ncu, 512], F32, tag="rsb")
                nc.vector.tensor_scalar(rsb[:], cps[:], b2c[:], 0.0, ALU.add, ALU.max)
                tlsb = fmp.tile([NSLOT * VIRT, 512], F32, tag="tlsb")
                nc.scalar.activation(tlsb[:], pps[:], AF.Tanh, bias=b2ph[:], scale=0.5)
                st[mm].update(esb=esb, rsb=rsb, tlsb=tlsb)

            def S3(mm):
                s, m = divmod(mm, N_MACRO)
                gm = m * G_MACRO
                esb, rsb, tlsb = st[mm]["esb"], st[mm]["rsb"], st[mm]["tlsb"]
                tcp = ps_tc.tile([128, 4, NSLOT * ncu], F32, tag="tcp")
                tpp = ps_tp.tile([128, 4, NSLOT * VIRT], F32, tag="tpp")
                for b in range(4):
                    nc.tensor.matmul(
                        tcp[:, b, :], esb[:, 128 * b:128 * (b + 1)],
                        ident[0:NSLOT * ncu, 0:NSLOT * ncu],
                        start=True, stop=False, is_transpose=True,
                    )
                    nc.tensor.matmul(
                        tcp[:, b, :], rsb[:, 128 * b:128 * (b + 1)],
                        ident[0:NSLOT * ncu, 0:NSLOT * ncu],
                        start=False, stop=True, is_transpose=True,
                    )
                    nc.tensor.transpose(
                        tpp[:, b, :], tlsb[:, 128 * b:128 * (b + 1)],
                        ident[0:NSLOT * VIRT, 0:NSLOT * VIRT],
                    )
                nc.scalar.copy(
                    sup[s]["tcsb"][:, gm:gm + G_MACRO, :].rearrange("p a b -> p (a b)"),
                    tcp[:].rearrange("p a b -> p (a b)"),
                )
                nc.scalar.copy(
                    sup[s]["tpsb"][:, gm:gm + G_MACRO, :].rearrange("p a b -> p (a b)"),
                    tpp[:].rearrange("p a b -> p (a b)"),
                )
                del st[mm]

            def make_phase_b(s):
                xu, tcsb, tpsb = sup[s]["xu"], sup[s]["tcsb"], sup[s]["tpsb"]
                ot = mpp.tile([128, G_SUPER, VIRT], F32, tag="ot")

                def xpl(i):
                    return xu[:, :, 7 + i]

                def tpl(f):
                    return xpl(f) if f < 4 else xu[:, :, int(tidc[f - 4])]

                def ichain(i, lo=0, hi=G_SUPER):
                    n = hi - lo
                    prs = []
                    for j in range(5):
                        t_ap = tpl(int(ti[i, j]))[:, lo:hi]
                        c_ap = tcsb[:, lo:hi, int(ai[i, j])]
                        d = mpp.tile([128, G_SUPER], F32, tag=f"d{j % 3}")
                        nc.gpsimd.tensor_tensor(d[:, :n], t_ap, xpl(i)[:, lo:hi], ALU.subtract)
                        pr = mpp.tile([128, G_SUPER], F32, tag=f"pr{j}")
                        nc.vector.tensor_tensor(pr[:, :n], d[:, :n], c_ap, ALU.mult)
                        prs.append(pr)
                    s01 = mpp.tile([128, G_SUPER], F32, tag="s01")
                    nc.vector.tensor_tensor(s01[:, :n], prs[0][:, :n], prs[1][:, :n], ALU.add)
                    s23 = mpp.tile([128, G_SUPER], F32, tag="s23")
                    nc.vector.tensor_tensor(s23[:, :n], prs[2][:, :n], prs[3][:, :n], ALU.add)
                    acc = mpp.tile([128, G_SUPER], F32, tag="acc")
                    nc.vector.tensor_tensor(acc[:, :n], s01[:, :n], s23[:, :n], ALU.add)
                    nc.vector.tensor_tensor(acc[:, :n], acc[:, :n], prs[4][:, :n], ALU.add)
                    z = mpp.tile([128, G_SUPER], F32, tag="z")
                    nc.vector.scalar_tensor_tensor(
                        z[:, :n], tpsb[:, lo:hi, i], 0.5, acc[:, :n], ALU.mult, ALU.add)
                    ec = float(ecap[i])
                    nc.vector.tensor_scalar(
                        ot[:, lo:hi, i], z[:, :n], ec, 0.5 * ec, ALU.mult, ALU.add)
                    nc.vector.tensor_scalar(
                        ot[:, lo:hi, i], ot[:, lo:hi, i], 1.0, -1.0, ALU.min, ALU.max)

                def outdma(lo=0, hi=G_SUPER):
                    g0 = s * G_SUPER
                    nc.gpsimd.dma_start(
                        out_v[:, g0 + lo:g0 + hi, :], ot[:, lo:hi, :])

                return ichain, outdma

            # slot -> deferred phase-B emissions.
            # B(s) (s < last) is ready after slot 8s+10; spread its 4 chains +
            # DMA over slots 8s+11..8s+15.  B(last): half0 ready after slot
            # 8(L)+3+3; spread at slots TOT..TOT+2; half1 + DMA at the end.
            b_emit = {}
            for s in range(N_SUPER - 1):
                ic_od = [None]
                for g in range(VIRT):
                    b_emit.setdefault(8 * s + 11 + g, []).append(("chain", s, g, 0, G_SUPER))
                b_emit.setdefault(8 * s + 15, []).append(("dma", s, 0, G_SUPER))

            L = N_SUPER - 1
            H = G_SUPER // 2
            for i in range(VIRT):
                b_emit.setdefault(8 * L + 7 + i, []).append(("chain", L, i, 0, H))

            b_funcs = {}

            def run_b(slot):
                for item in b_emit.get(slot, ()):
                    kind, s = item[0], item[1]
                    if s not in b_funcs:
                        b_funcs[s] = make_phase_b(s)
                    ic, od = b_funcs[s]
                    if kind == "chain":
                        _, _, i, lo, hi = item
                        ic(i, lo, hi)
                    else:
                        _, _, lo, hi = item
                        od(lo, hi)

            for slot in range(TOT + 3):
                if slot < TOT:
                    s0, m0 = divmod(slot, N_MACRO)
                    ensure_super(s0)
                    S0(slot)
                if 0 <= slot - 1 < TOT:
                    S1(slot - 1)
                if 0 <= slot - 2 < TOT:
                    S2(slot - 2)
                if 0 <= slot - 3 < TOT:
                    S3(slot - 3)
                run_b(slot)

            # tail: last super-chunk's second half + output DMA
            ic, od = b_funcs[L]
            for i in range(VIRT):
                ic(i, H, G_SUPER)
            od(0, G_SUPER)

    return nc


def _prep_weights(cw1, cb1, cw2, cb2, pw1, pb1, pw2, pb2, ncu=NCOND):
    w1 = np.concatenate([np.asarray(cw1), np.asarray(pw1)], axis=1)  # [11, 6]
    b1 = np.concatenate([np.asarray(cb1), np.asarray(pb1)])          # [6]
    w2c = np.zeros((NHID, ncu), np.float32)
    w2c[0:2, :] = np.asarray(cw2)[:, :ncu]
    w2p = np.zeros((NHID, VIRT), np.float32)
    w2p[2:6, :] = np.asarray(pw2)

    def blockdiag(w):
        r, c = w.shape
        out = np.zeros((NSLOT * r, NSLOT * c), np.float32)
        for j in range(NSLOT):
            out[j * r:(j + 1) * r, j * c:(j + 1) * c] = w
        return out

    w1blk = blockdiag(w1.astype(np.float32))
    w2cblk = blockdiag(w2c)
    w2pblk = blockdiag(w2p)
    b1blk = np.tile(np.asarray(b1, np.float32), NSLOT)[:, None]
    b2cblk = np.tile(np.asarray(cb2, np.float32)[:ncu], NSLOT)[:, None]
    b2pblkh = 0.5 * np.tile(np.asarray(pb2, np.float32), NSLOT)[:, None]
    return w1blk, b1blk, w2cblk, b2cblk, w2pblk, b2pblkh


def kernel(t, x, u, caps, cw1, cb1, cw2, cb2, pw1, pb1, pw2, pb2,
           temps_indexer, adj_idx, temp_idcs):
    global LAST_RESULTS
    t = np.asarray(t)
    x = np.asarray(x, np.float32)
    u = np.asarray(u, np.float32)
    step = int(round(float(t) * 2))
    u2 = np.ascontiguousarray(u[step])                      # [B, 7]

    ti = np.asarray(temps_indexer, np.int64)
    ai = np.asarray(adj_idx, np.int64)
    tidc = np.asarray(temp_idcs, np.int64)
    ecap = np.exp(np.asarray(caps, np.float64))

    ncu = int(ai.max()) + 1  # conducts beyond the referenced ones are dead
    key = (step, tuple(ti.ravel()), tuple(ai.ravel()), tuple(tidc.ravel()),
           tuple(np.asarray(ecap).ravel().tolist()))
    if key not in _CACHE:
        _CACHE.clear()
        _CACHE[key] = _build_bass(ti, ai, tidc, ecap, ncu)
    nc = _CACHE[key]

    w1blk, b1blk, w2cblk, b2cblk, w2pblk, b2pblkh = _prep_weights(
        cw1, cb1, cw2, cb2, pw1, pb1, pw2, pb2, ncu)
    ident = np.eye(128, dtype=np.float32)

    in_maps = []
    for c in range(N_CORES):
        lo = c * B_CORE
        xs = np.zeros((NP_CORE, 4), np.float32)
        us = np.zeros((NP_CORE, 7), np.float32)
        xs[:B_CORE] = x[lo:lo + B_CORE]
        us[:B_CORE] = u2[lo:lo + B_CORE]
        in_maps.append({
            "u2": us, "xin": xs, "ident": ident,
            "w1blk": w1blk, "b1blk": b1blk,
            "w2cblk": w2cblk, "b2cblk": b2cblk,
            "w2pblk": w2pblk, "b2pblkh": b2pblkh,
        })

    import time
    trace = bool(int(os.environ.get("KERNEL_TRACE", "0")))
    t0 = time.time()
    res = bass_utils.run_bass_kernel_spmd(
        nc, in_maps, core_ids=list(range(N_CORES)), trace=trace)
    global LAST_WALL
    LAST_WALL = time.time() - t0
    LAST_RESULTS = res

    out = np.empty((B_TOTAL, 4), np.float32)
    for c in range(N_CORES):
        out[c * B_CORE:(c + 1) * B_CORE] = np.asarray(res.results[c]["out"])[:B_CORE]
    return out
